# revision 6
# baseline (speedup 1.0000x reference)
"""Trainium2 Bass kernel for BasicBlockIMCFlow (quantized ResNet basic block).

Math (exact integer arithmetic; fp32/int16/fp8 carriers):
  t    = sat_i16(rne(x*256))                      (== reference to_int16)
  q1   = min(relu_rne_i16(t/1024 + 2^-11), 15)    (== clip(floor((t+512)/1024),0,15))
  h1   = conv3x3(q1, w1)
  q2   = clip(rne_i16(h1*s1/2048 + (2*b1+1)/4096), 0, 15)
                                                  (== clip(floor((h1*s1+b1+1024)/2048),0,15))
  h2   = conv3x3(q2, w2)
  oi   = sat_i16(h2*s2 + b2 + t)                  (== to_int16(h2*s2+b2+x_int))
  out  = oi / 256  (host side)

Engine float->int16 conversions round-to-nearest-even and saturate (verified
on hardware) == jnp.round + int16 clip; the 2^-11 / (2b+1)/4096 bias guards
turn round-to-nearest into the required floor with no representable ties.

Convs are fp8 DoubleRow matmuls (2 k-tiles per pass, 2x contraction): the two
images of a pair are stacked on the 128 partitions with block-diagonal
weights; tap pairs are fed from one zero-padded [66,66] fp8 buffer via 4D
overlapping access patterns. Per 4-output-row chunk: 4 DoubleRow passes
(8 taps) + 1 regular pass (tap (2,2)) = 5 matmuls, ~122 ns/matmul measured.

Schedule: software-pipelined sweeps. Sweep p runs conv2(pair p) interleaved
with conv1(pair p+1) on the PE while stage A (load+quantize) of pair p+2 runs
on Act/DVE, emitted mid-sweep so psum post-ops are not head-of-line blocked.
gpsimd only triggers weight DMAs (its compute is ~20x slower than DVE).

Data parallel: batch 64 -> 8 images/core; output is DMA'd as int16 and
divided by 256 on the host (exact).
"""

import os

import numpy as np

_CACHE = {}

B, C, H, W = 64, 64, 64, 64
HW = H * W                    # 4096
PW = W + 2                    # 66
N_CORES = 8
IMG_PER_CORE = B // N_CORES   # 8
PAIRS = IMG_PER_CORE // 2     # 4
CHUNK_ROWS = 4                # output rows per matmul chunk
NCHUNK = H // CHUNK_ROWS      # 16
CHUNK_N = CHUNK_ROWS * W      # 256
GEN_CHUNKS = 4                # chunks per psum generation tile
NGEN = NCHUNK // GEN_CHUNKS   # 4
GEN_N = GEN_CHUNKS * CHUNK_N  # 1024
NQ = 4                        # stage A quarters
Q_N = HW // NQ                # 1024
Q_ROWS = H // NQ              # 16

TAP_PAIRS = [((0, 0), (0, 1)), ((1, 1), (1, 2)), ((2, 0), (2, 1)),
             ((0, 2), (1, 0))]
TAP_SINGLE = (2, 2)


def _build_nc():
    import concourse.bacc as bacc
    import concourse.tile as tile
    import concourse.mybir as mybir
    from contextlib import ExitStack

    f32 = mybir.dt.float32
    i16 = mybir.dt.int16
    fp8 = mybir.dt.float8e4
    Alu = mybir.AluOpType
    Act = mybir.ActivationFunctionType
    DR = mybir.MatmulPerfMode.DoubleRow

    nc = bacc.Bacc()

    x_d = nc.dram_tensor("x", [IMG_PER_CORE, C, HW], f32, kind="ExternalInput")
    w1_d = nc.dram_tensor("w1t", [128, 9 * 128], fp8, kind="ExternalInput")
    w2_d = nc.dram_tensor("w2t", [128, 9 * 128], fp8, kind="ExternalInput")
    pp_d = nc.dram_tensor("pp", [128, 6], f32, kind="ExternalInput")
    out_d = nc.dram_tensor("out", [IMG_PER_CORE, C, HW], i16,
                           kind="ExternalOutput")

    with tile.TileContext(nc) as tc:
        with ExitStack() as ctx:
            singles = ctx.enter_context(tc.tile_pool(name="singles", bufs=1))
            xsp = ctx.enter_context(tc.tile_pool(name="xsp", bufs=2))
            tp = ctx.enter_context(tc.tile_pool(name="tp", bufs=3))
            ap16 = ctx.enter_context(tc.tile_pool(name="ap16", bufs=2))
            qp1p = ctx.enter_context(tc.tile_pool(name="qp1p", bufs=2))
            qp2p = ctx.enter_context(tc.tile_pool(name="qp2p", bufs=2))
            g2p = ctx.enter_context(tc.tile_pool(name="g2p", bufs=3))
            up = ctx.enter_context(tc.tile_pool(name="up", bufs=3))
            otp = ctx.enter_context(tc.tile_pool(name="otp", bufs=4))
            ps1 = ctx.enter_context(tc.tile_pool(name="ps1", bufs=2,
                                                 space="PSUM"))
            ps2 = ctx.enter_context(tc.tile_pool(name="ps2", bufs=2,
                                                 space="PSUM"))

            w1b = singles.tile([128, 9, 128], fp8, tag="w1b")
            w1r = w1_d.rearrange("p (t m) -> p t m", m=128)
            nc.gpsimd.dma_start(out=w1b[:, 0:5, :], in_=w1r[:, 0:5, :])
            nc.scalar.dma_start(out=w1b[:, 5:9, :], in_=w1r[:, 5:9, :])
            pp = singles.tile([128, 6], f32, tag="pp")
            nc.sync.dma_start(out=pp, in_=pp_d[:])
            w2b = singles.tile([128, 9, 128], fp8, tag="w2b")
            nc.gpsimd.dma_start(out=w2b,
                                in_=w2_d.rearrange("p (t m) -> p t m", m=128))
            sB, bB = pp[:, 0:1], pp[:, 1:2]
            sC, bC = pp[:, 2:3], pp[:, 3:4]
            eps11 = pp[:, 4:5]   # 2^-11
            zero_c = pp[:, 5:6]  # 0.0

            def conv_rhs(qp, r, kyA, kxA, delta):
                full = qp[:, :, :]
                ap = full.copy()
                VP = type(ap.ap)
                ap.ap = VP([[full.ap[0][0], 128], [delta, 2],
                            [PW, CHUNK_ROWS], [1, W]])
                ap.offset = full.offset + (r + kyA) * PW + kxA
                return ap

            def conv_chunk(ps_tile, col0, wb, qp, j):
                """5 passes (4 DoubleRow + 1 single-tap) for rows 4j..4j+3."""
                r = j * CHUNK_ROWS
                dst = ps_tile[:, col0:col0 + CHUNK_N]
                for g, ((kyA, kxA), (kyB, kxB)) in enumerate(TAP_PAIRS):
                    delta = (kyB - kyA) * PW + (kxB - kxA)
                    nc.tensor.matmul(
                        dst, wb[:, 2 * g:2 * g + 2, :],
                        conv_rhs(qp, r, kyA, kxA, delta),
                        start=(g == 0), stop=False, perf_mode=DR)
                ky, kx = TAP_SINGLE
                nc.tensor.matmul(
                    dst, wb[:, 8, :],
                    qp[:, r + ky:r + ky + CHUNK_ROWS, kx:kx + W],
                    start=False, stop=True)

            def conv_chunk_pair(ps_tile, col0, wb, qp, j):
                conv_chunk(ps_tile, col0, wb, qp, j)
                conv_chunk(ps_tile, col0 + CHUNK_N, wb, qp, j + 1)

            # ---- per-pair state and stage helpers ----
            def dma_x(st, spread=False):
                xs = xsp.tile([128, HW], f32, tag="xs")
                st["xs"] = xs
                x_pair = x_d[2 * st["p"]:2 * st["p"] + 2, :, :].rearrange(
                    "b c n -> (b c) n")
                engines = ([nc.sync, nc.scalar, nc.gpsimd, nc.sync]
                           if spread else [nc.sync] * NQ)
                for q in range(NQ):
                    cs = slice(q * Q_N, (q + 1) * Q_N)
                    engines[q].dma_start(out=st["xs"][:, cs],
                                         in_=x_pair[:, cs])

            def stage_a_quarter(st, q, split=1):
                for h in range(split):
                    n = Q_N // split
                    cs = slice(q * Q_N + h * n, q * Q_N + (h + 1) * n)
                    # t = sat_i16(rne(256 x))
                    nc.scalar.activation(out=st["t16"][:, cs],
                                         in_=st["xs"][:, cs],
                                         func=Act.Identity, bias=zero_c,
                                         scale=256.0)
                    # a = rne_i16(relu(t/1024 + 2^-11)) in [0, 32]
                    nc.scalar.activation(out=st["a16"][:, cs],
                                         in_=st["t16"][:, cs], func=Act.Relu,
                                         bias=eps11, scale=2.0 ** -10)
                    # q1 = min(a, 15) -> fp8 strided into padded interior
                    rows = Q_ROWS // split
                    r0 = q * Q_ROWS + h * rows
                    dq = st["qp1"][:, 1 + r0:1 + r0 + rows, 1:W + 1]
                    nc.vector.tensor_scalar(out=dq, in0=st["a16"][:, cs],
                                            scalar1=15, scalar2=None,
                                            op0=Alu.min)

            def borders(qp):
                nc.vector.memset(qp[:, 0, :], 0.0)
                nc.vector.memset(qp[:, H + 1, :], 0.0)
                nc.vector.memset(qp[:, 1:H + 1, 0], 0.0)
                nc.vector.memset(qp[:, 1:H + 1, PW - 1], 0.0)

            def conv1_gen(st, g):
                pst = ps1.tile([128, GEN_N], f32, tag="ps1")
                for k in range(0, GEN_CHUNKS, 2):
                    conv_chunk_pair(pst, k * CHUNK_N, w1b, st["qp1"],
                                    g * GEN_CHUNKS + k)
                # g2 = rne_i16(h1*s1/2048 + (2 b1 + 1)/4096)
                g2 = g2p.tile([128, GEN_N], i16, tag="g2")
                nc.vector.tensor_scalar(out=g2, in0=pst, scalar1=sB,
                                        scalar2=bB, op0=Alu.mult, op1=Alu.add)
                # q2 = clip(g2, 0, 15) -> fp8 strided interior rows
                r0 = g * GEN_CHUNKS * CHUNK_ROWS
                dq = st["qp2"][:, 1 + r0:1 + r0 + GEN_CHUNKS * CHUNK_ROWS,
                               1:W + 1]
                nc.vector.tensor_scalar(out=dq, in0=g2, scalar1=0, scalar2=15,
                                        op0=Alu.max, op1=Alu.min)

            def conv2_gen(st, g, split=1):
                pst = ps2.tile([128, GEN_N], f32, tag="ps2")
                for k in range(0, GEN_CHUNKS, 2):
                    conv_chunk_pair(pst, k * CHUNK_N, w2b, st["qp2"],
                                    g * GEN_CHUNKS + k)
                i0 = st["p"] * 2
                u = up.tile([128, GEN_N], f32, tag="u")
                ot = otp.tile([128, GEN_N], i16, tag="ot")
                for h in range(split):
                    n = GEN_N // split
                    hs = slice(h * n, (h + 1) * n)
                    # u = h2*s2 + b2
                    nc.scalar.activation(out=u[:, hs], in_=pst[:, hs],
                                         func=Act.Identity, bias=bC, scale=sC)
                    # oi = sat_i16(t + u)
                    gs = slice(g * GEN_N + h * n, g * GEN_N + (h + 1) * n)
                    nc.vector.scalar_tensor_tensor(out=ot[:, hs],
                                                   in0=st["t16"][:, gs],
                                                   scalar=0.0, in1=u[:, hs],
                                                   op0=Alu.add, op1=Alu.add)
                    od = out_d[i0:i0 + 2, :, gs].rearrange("b c n -> (b c) n")
                    nc.sync.dma_start(out=od, in_=ot[:, hs])

            def new_state(p):
                t16 = tp.tile([128, HW], i16, tag="t16")
                a16 = ap16.tile([128, HW], i16, tag="a16")
                qp1 = qp1p.tile([128, H + 2, PW], fp8, tag="qp1")
                qp2 = qp2p.tile([128, H + 2, PW], fp8, tag="qp2")
                return {"p": p, "t16": t16, "a16": a16, "qp1": qp1,
                        "qp2": qp2}

            # ---- prologue: stage A(0) fully, then conv1(0) with stage A(1) ----
            states = [new_state(p) for p in range(PAIRS)]
            s0, s1 = states[0], states[1]
            dma_x(s0, spread=True)
            borders(s0["qp1"])
            borders(s0["qp2"])
            for q in range(NQ):
                stage_a_quarter(s0, q, split=2)
            dma_x(s1)
            borders(s1["qp1"])
            borders(s1["qp2"])
            for g in range(NGEN):
                conv1_gen(s0, g)
                stage_a_quarter(s1, g)

            # ---- sweeps ----
            for p in range(PAIRS):
                cur = states[p]
                nx1 = states[p + 1] if p + 1 < PAIRS else None
                nx2 = states[p + 2] if p + 2 < PAIRS else None
                if nx2 is not None:
                    dma_x(nx2)
                    borders(nx2["qp1"])
                last = (p == PAIRS - 1)
                for g in range(NGEN):
                    conv2_gen(cur, g, split=2 if (last and g == NGEN - 1)
                              else 1)
                    if nx1 is not None:
                        conv1_gen(nx1, g)
                    if nx2 is not None:
                        stage_a_quarter(nx2, g)
                # qp2(p+2) shares SBUF with qp2(p): emit its border zeroing
                # after conv2(p) so the WAR wait does not block the DVE queue
                if nx2 is not None:
                    borders(nx2["qp2"])

    nc.compile()
    return nc


def _get_nc():
    if "nc" not in _CACHE:
        _CACHE["nc"] = _build_nc()
    return _CACHE["nc"]


def _prep_host_inputs(inputs):
    import concourse.mybir as mybir

    fp8np = mybir.dt.np(mybir.dt.float8e4)

    x = np.ascontiguousarray(inputs["x"], dtype=np.float32).reshape(B, C, HW)

    def wprep(w):
        wt = np.ascontiguousarray(w, dtype=np.float32).reshape(C, C, 3, 3)
        wt = wt.transpose(1, 0, 2, 3)                  # [in, out, ky, kx]
        taps = [kk for pair in TAP_PAIRS for kk in pair] + [TAP_SINGLE]
        out = np.zeros((128, 9, 128), np.float32)
        for t, (ky, kx) in enumerate(taps):
            out[0:64, t, 0:64] = wt[:, :, ky, kx]
            out[64:128, t, 64:128] = wt[:, :, ky, kx]
        return np.ascontiguousarray(out.reshape(128, 9 * 128).astype(fp8np))

    w1t = wprep(inputs["w1"])
    w2t = wprep(inputs["w2"])

    s1 = np.asarray(inputs["bn1_scale"], dtype=np.float64)
    b1 = np.asarray(inputs["bn1_bias"], dtype=np.float64)
    s2 = np.asarray(inputs["bn2_scale"], dtype=np.float64)
    b2 = np.asarray(inputs["bn2_bias"], dtype=np.float64)
    # all exact dyadic rationals -> float32 conversion is exact
    sB = (s1 * 2.0 ** -11).astype(np.float32)
    bB = ((2.0 * b1 + 1.0) * 2.0 ** -12).astype(np.float32)
    sC = s2.astype(np.float32)
    bC = b2.astype(np.float32)
    eps = np.full(64, 2.0 ** -11, dtype=np.float32)
    zer = np.zeros(64, dtype=np.float32)
    pp = np.stack([sB, bB, sC, bC, eps, zer], axis=1)      # [64, 6]
    pp = np.ascontiguousarray(np.concatenate([pp, pp], axis=0))  # [128, 6]

    return x, w1t, w2t, pp


def kernel(**inputs):
    from concourse.bass_utils import run_bass_kernel_spmd

    x, w1t, w2t, pp = _prep_host_inputs(inputs)
    nc = _get_nc()
    in_maps = []
    for i in range(N_CORES):
        shard = np.ascontiguousarray(x[i * IMG_PER_CORE:(i + 1) * IMG_PER_CORE])
        in_maps.append({"x": shard, "w1t": w1t, "w2t": w2t, "pp": pp})

    trace = bool(int(os.environ.get("KERNEL_TRACE", "0")))
    res = run_bass_kernel_spmd(nc, in_maps, core_ids=list(range(N_CORES)),
                               trace=trace)
    _CACHE["last_results"] = res
    out = np.concatenate([r["out"] for r in res.results], axis=0)
    return (out.reshape(B, C, H, W).astype(np.float32)) / 256.0
